# revision 1
# baseline (speedup 1.0000x reference)
"""LIIF-net Trainium2 kernel.

Sharding: 8 cores = batch (4) x query-halves (2). Each core:
  - precomputes z[p, c] = unfold3x3(feat) @ w0[:576] + ramp @ w0[576:578] + b0
    for all 4096 pixels of its batch (channel-major in SBUF),
  - per 512-query chunk and per ensemble branch: ap_gather of z columns,
    h0 = relu(tail_mm + z_gather) in PSUM (tail = per-query rank-4 matmul,
    z added via identity matmul), then 3x (256x256 matmul + fused relu+bias),
  - ensemble weights (host-computed, exactly replicating the reference's
    nearest-index/area math in fp32) are multiplied into h3, so the final
    3-wide layer accumulates all 4 branches in a single PSUM group.
"""

import numpy as np
from contextlib import ExitStack

import concourse.bass as bass
import concourse.bacc as bacc
import concourse.mybir as mybir
import concourse.tile as tile
from concourse import library_config
from concourse.bass_utils import run_bass_kernel_spmd

F32 = mybir.dt.float32
F32R = mybir.dt.float32r
I16 = mybir.dt.int16

B, C, H, W = 4, 64, 64, 64
Q = 192 * 192
HID = 256
NCORES = 8
QSH = 2
NQ = Q // QSH          # 18432 queries per core
RT = 512               # queries per chunk (fp32 matmul free-dim max)
NPIX = H * W           # 4096
EPS_SHIFT = 1e-6
EPS_CLAMP = 1e-6

_prog_cache = {}


def build_program(nq=NQ):
    nchunk = nq // RT
    nc = bacc.Bacc()

    feat_d = nc.declare_dram_parameter("feat", [C, NPIX], F32R, isOutput=False)
    w0r_d = nc.declare_dram_parameter("w0r", [9, C, HID], F32R, isOutput=False)
    w0t_d = nc.declare_dram_parameter("w0t", [4, HID], F32R, isOutput=False)
    ab_d = nc.declare_dram_parameter("ab", [2, NPIX], F32R, isOutput=False)
    w1_d = nc.declare_dram_parameter("w1", [HID, HID], F32R, isOutput=False)
    w2_d = nc.declare_dram_parameter("w2", [HID, HID], F32R, isOutput=False)
    w3_d = nc.declare_dram_parameter("w3", [HID, HID], F32R, isOutput=False)
    w4_d = nc.declare_dram_parameter("w4", [HID, 3], F32R, isOutput=False)
    b0_d = nc.declare_dram_parameter("b0", [HID, 1], F32, isOutput=False)
    b1_d = nc.declare_dram_parameter("b1", [HID, 1], F32, isOutput=False)
    b2_d = nc.declare_dram_parameter("b2", [HID, 1], F32, isOutput=False)
    b3_d = nc.declare_dram_parameter("b3", [HID, 1], F32, isOutput=False)
    b4_d = nc.declare_dram_parameter("b4", [3, 1], F32, isOutput=False)
    tailf_d = nc.declare_dram_parameter("tailf", [4, nq], F32R, isOutput=False)
    wv_d = nc.declare_dram_parameter("wv", [4, nq], F32, isOutput=False)
    idx_d = nc.declare_dram_parameter("idx", [128, 4 * nq // 16], I16, isOutput=False)
    ident_d = nc.declare_dram_parameter("ident", [128, 128], F32R, isOutput=False)
    out_d = nc.declare_dram_parameter("out_t", [3, nq], F32, isOutput=True)

    with tile.TileContext(nc) as tc, \
         tc.tile_pool(name="persist", bufs=1) as persist:
        # --- persistent SBUF state ---
        z_sb = [persist.tile([128, NPIX], F32, name=f"z{m}", tag=f"z{m}") for m in range(2)]
        # w1..w3 stored as (128, 2*HID): cols kk*HID + mm*128 give lhsT tiles
        w_sb = []
        for wd, nm in ((w1_d, "w1"), (w2_d, "w2"), (w3_d, "w3")):
            t = persist.tile([128, 2 * HID], F32R, tag=nm)
            for kk in range(2):
                nc.sync.dma_start(
                    out=t[:, kk * HID:(kk + 1) * HID],
                    in_=wd[kk * 128:(kk + 1) * 128, :],
                )
            w_sb.append(t)
        w4_sb = persist.tile([128, 6], F32R, name="w4", tag="w4")
        for kk in range(2):
            nc.sync.dma_start(
                out=w4_sb[:, kk * 3:(kk + 1) * 3],
                in_=w4_d[kk * 128:(kk + 1) * 128, :],
            )
        w0t_sb = persist.tile([4, HID], F32R, name="w0t", tag="w0t")
        nc.sync.dma_start(out=w0t_sb, in_=w0t_d[:, :])
        ident_sb = persist.tile([128, 128], F32R, name="ident", tag="ident")
        nc.sync.dma_start(out=ident_sb, in_=ident_d[:, :])
        b_sb = {}
        for bd, nm in ((b0_d, "b0"), (b1_d, "b1"), (b2_d, "b2"), (b3_d, "b3")):
            t = persist.tile([128, 2], F32, tag=nm)
            for m in range(2):
                nc.sync.dma_start(
                    out=t[:, m:m + 1], in_=bd[m * 128:(m + 1) * 128, :]
                )
            b_sb[nm] = t
        b4_sb = persist.tile([3, 1], F32, name="b4", tag="b4")
        nc.sync.dma_start(out=b4_sb, in_=b4_d[:, :])
        tailf_sb = persist.tile([4, nq], F32R, name="tailf", tag="tailf")
        nc.sync.dma_start(out=tailf_sb, in_=tailf_d[:, :])
        idx_sb = persist.tile([128, 4 * nq // 16], I16, name="idx", tag="idx")
        nc.sync.dma_start(out=idx_sb, in_=idx_d[:, :])

        # --- precompute z tables ---
        with tc.tile_pool(name="pre", bufs=1) as pre, \
             tc.tile_pool(name="prepsum", bufs=4, space="PSUM") as prepsum:
            featpad = pre.tile([C, 66 * 66], F32R, name="featpad", tag="featpad")
            nc.vector.memset(featpad[:, :].bitcast(mybir.dt.uint32), 0)
            fp_v = featpad[:].rearrange("p (r c) -> p r c", r=66, c=66)
            nc.sync.dma_start(out=fp_v[:, 1:65, 1:65], in_=feat_d[:, :])
            w0r_sb = pre.tile([C, 9 * HID], F32R, name="w0r", tag="w0r")
            nc.sync.dma_start(
                out=w0r_sb[:, :].rearrange("p (k m) -> p k m", k=9),
                in_=w0r_d[:, :, :].rearrange("k c m -> c k m"),
            )
            ab_sb = pre.tile([2, NPIX], F32R, name="ab", tag="ab")
            nc.sync.dma_start(out=ab_sb, in_=ab_d[:, :])

            for m in range(2):
                for cchunk in range(NPIX // RT):
                    ps = prepsum.tile([128, RT], F32, name="preps", tag="preps")
                    r0 = cchunk * 8
                    for k in range(9):
                        kh, kw = k // 3, k % 3
                        rhs = fp_v[:, r0 + kh:r0 + kh + 8, kw:kw + 64]
                        nc.tensor.matmul(
                            ps,
                            w0r_sb[:, k * HID + m * 128: k * HID + m * 128 + 128],
                            rhs,
                            start=(k == 0),
                            stop=False,
                        )
                    nc.tensor.matmul(
                        ps,
                        w0t_sb[0:2, m * 128:m * 128 + 128],
                        ab_sb[:, cchunk * RT:(cchunk + 1) * RT],
                        start=False,
                        stop=True,
                    )
                    nc.scalar.activation(
                        z_sb[m][:, cchunk * RT:(cchunk + 1) * RT],
                        ps,
                        mybir.ActivationFunctionType.Identity,
                        bias=b_sb["b0"][:, m:m + 1],
                    )

        # --- main loop ---
        with tc.tile_pool(name="zg", bufs=3) as zgp, \
             tc.tile_pool(name="acts", bufs=3) as actp, \
             tc.tile_pool(name="wbp", bufs=3) as wbp, \
             tc.tile_pool(name="outp", bufs=3) as outp, \
             tc.tile_pool(name="pp", bufs=6, space="PSUM") as pp, \
             tc.tile_pool(name="predpp", bufs=2, space="PSUM") as predpp:
            for t in range(nchunk):
                f0 = t * RT
                tail_rhs = tailf_sb[:, f0:f0 + RT]
                pred_ps = predpp.tile([3, RT], F32, name="predps", tag="predps")
                for v in range(4):
                    zg = [zgp.tile([128, RT], F32, name=f"zg{m}", tag=f"zg{m}") for m in range(2)]
                    idx_ap = idx_sb[:, (v * nq + f0) // 16:(v * nq + f0) // 16 + RT // 16]
                    zgr = [zgp.tile([128, RT], F32R, name=f"zgr{m}", tag=f"zgr{m}") for m in range(2)]
                    for m in range(2):
                        nc.gpsimd.ap_gather(
                            zg[m], z_sb[m][:], idx_ap,
                            channels=128, num_elems=NPIX, d=1, num_idxs=RT,
                        )
                        nc.sync.dma_start(out=zgr[m], in_=zg[m].bitcast(F32R))
                    wb = wbp.tile([128, RT], F32, name="wb", tag="wb")
                    wv_ap = wv_d[:, :]
                    nc.sync.dma_start(
                        out=wb,
                        in_=bass.AP(
                            tensor=wv_ap.tensor,
                            offset=v * nq + f0,
                            ap=[[0, 128], [1, RT]],
                        ),
                    )
                    h0 = [actp.tile([128, RT], F32R, name=f"h0_{m}", tag=f"h0_{m}") for m in range(2)]
                    for m in range(2):
                        ps = pp.tile([128, RT], F32, name="ps", tag="ps")
                        nc.tensor.matmul(
                            ps, w0t_sb[:, m * 128:m * 128 + 128], tail_rhs,
                            start=True, stop=False,
                        )
                        nc.tensor.matmul(ps, ident_sb[:, :], zgr[m], start=False, stop=True)
                        nc.scalar.activation(
                            h0[m], ps, mybir.ActivationFunctionType.Relu
                        )
                    h1 = [actp.tile([128, RT], F32R, name=f"h1_{m}", tag=f"h1_{m}") for m in range(2)]
                    for m in range(2):
                        ps = pp.tile([128, RT], F32, name="ps", tag="ps")
                        for kk in range(2):
                            nc.tensor.matmul(
                                ps, w_sb[0][:, kk * HID + m * 128: kk * HID + m * 128 + 128],
                                h0[kk], start=(kk == 0), stop=(kk == 1),
                            )
                        nc.scalar.activation(
                            h1[m], ps, mybir.ActivationFunctionType.Relu,
                            bias=b_sb["b1"][:, m:m + 1],
                        )
                    h2 = [actp.tile([128, RT], F32R, name=f"h2_{m}", tag=f"h2_{m}") for m in range(2)]
                    for m in range(2):
                        ps = pp.tile([128, RT], F32, name="ps", tag="ps")
                        for kk in range(2):
                            nc.tensor.matmul(
                                ps, w_sb[1][:, kk * HID + m * 128: kk * HID + m * 128 + 128],
                                h1[kk], start=(kk == 0), stop=(kk == 1),
                            )
                        nc.vector.tensor_scalar(
                            h2[m], ps, b_sb["b2"][:, m:m + 1], 0.0,
                            mybir.AluOpType.add, mybir.AluOpType.max,
                        )
                    h3 = [actp.tile([128, RT], F32R, name=f"h3_{m}", tag=f"h3_{m}") for m in range(2)]
                    for m in range(2):
                        ps = pp.tile([128, RT], F32, name="ps", tag="ps")
                        for kk in range(2):
                            nc.tensor.matmul(
                                ps, w_sb[2][:, kk * HID + m * 128: kk * HID + m * 128 + 128],
                                h2[kk], start=(kk == 0), stop=(kk == 1),
                            )
                        # h3w = relu(ps + b3) * wb  (wb > 0) = max((ps+b3)*wb, 0)
                        nc.vector.scalar_tensor_tensor(
                            h3[m], ps, b_sb["b3"][:, m:m + 1], wb,
                            op0=mybir.AluOpType.add, op1=mybir.AluOpType.mult,
                        )
                        nc.vector.tensor_scalar_max(h3[m], h3[m], 0.0)
                    for m in range(2):
                        nc.tensor.matmul(
                            pred_ps, w4_sb[:, m * 3:m * 3 + 3], h3[m],
                            start=(v == 0 and m == 0), stop=(v == 3 and m == 1),
                            skip_group_check=True,
                        )
                pred_sb = outp.tile([3, RT], F32, name="pred", tag="pred")
                nc.scalar.activation(
                    pred_sb, pred_ps, mybir.ActivationFunctionType.Identity, bias=b4_sb
                )
                nc.sync.dma_start(out=out_d[:, f0:f0 + RT], in_=pred_sb)

    nc.finalize()
    return nc


def _host_prep(feat, coord, cell, w0, b0, w1, b1, w2, b2, w3, b3, w4, b4,
               nq=NQ):
    """Returns per-core input maps (list of 8 dicts)."""
    f32 = np.float32
    feat = np.ascontiguousarray(feat, f32)
    coord = np.ascontiguousarray(coord, f32)
    cell = np.ascontiguousarray(cell, f32)
    w0 = np.ascontiguousarray(w0, f32)

    w0r = np.ascontiguousarray(w0[:576].reshape(C, 9, HID).transpose(1, 0, 2))
    w0t = np.ascontiguousarray(w0[576:580])
    pix = np.arange(NPIX)
    ab = np.stack([63.0 - 2.0 * (pix // W), 63.0 - 2.0 * (pix % W)]).astype(f32)
    ident = np.eye(128, dtype=f32)

    shared = dict(
        w0r=w0r, w0t=w0t, ab=ab,
        w1=np.ascontiguousarray(w1, f32), w2=np.ascontiguousarray(w2, f32),
        w3=np.ascontiguousarray(w3, f32), w4=np.ascontiguousarray(w4, f32),
        b0=np.ascontiguousarray(b0, f32).reshape(HID, 1),
        b1=np.ascontiguousarray(b1, f32).reshape(HID, 1),
        b2=np.ascontiguousarray(b2, f32).reshape(HID, 1),
        b3=np.ascontiguousarray(b3, f32).reshape(HID, 1),
        b4=np.ascontiguousarray(b4, f32).reshape(3, 1),
        ident=ident,
    )

    # host index/area math, replicating the reference's f32 ops exactly
    cx = coord[:, :, 0]
    cy = coord[:, :, 1]
    rx = 1.0 / H
    ry = 1.0 / W
    idx_all = []
    area_all = []
    for vx in (-1.0, 1.0):
        for vy in (-1.0, 1.0):
            cxs = np.clip(cx + vx * rx + EPS_SHIFT, -1 + EPS_CLAMP, 1 - EPS_CLAMP).astype(f32)
            cys = np.clip(cy + vy * ry + EPS_SHIFT, -1 + EPS_CLAMP, 1 - EPS_CLAMP).astype(f32)
            iy = np.clip(np.floor(((cxs + 1) * H - 1) * 0.5 + 0.5), 0, H - 1).astype(np.int32)
            ix = np.clip(np.floor(((cys + 1) * W - 1) * 0.5 + 0.5), 0, W - 1).astype(np.int32)
            qcx = ((iy.astype(f32) + 0.5) * (2.0 / H) - 1.0).astype(f32)
            qcy = ((ix.astype(f32) + 0.5) * (2.0 / W) - 1.0).astype(f32)
            rel_x = ((cx - qcx) * H).astype(f32)
            rel_y = ((cy - qcy) * W).astype(f32)
            idx_all.append(iy * W + ix)
            area_all.append((np.abs(rel_x * rel_y) + 1e-9).astype(f32))
    tot = (area_all[0] + area_all[1] + area_all[2] + area_all[3]).astype(f32)
    order = (3, 2, 1, 0)
    wv_all = [(area_all[order[v]] / tot).astype(f32) for v in range(4)]

    tail_all = np.stack(
        [cx * H, cy * W, cell[:, :, 0] * H, cell[:, :, 1] * W], axis=1
    ).astype(f32)  # (B, 4, Q)

    in_maps = []
    for core in range(B * QSH):
        b, half = core // QSH, core % QSH
        qs = slice(half * nq, (half + 1) * nq)
        idx_w = np.concatenate(
            [
                np.tile(
                    idx_all[v][b, qs].astype(np.int16).reshape(nq // 16, 16).T,
                    (8, 1),
                )
                for v in range(4)
            ],
            axis=1,
        )  # (128, 4*nq/16)
        m = dict(shared)
        m["feat"] = np.ascontiguousarray(feat[b].reshape(C, NPIX))
        m["tailf"] = np.ascontiguousarray(tail_all[b, :, qs])
        m["wv"] = np.ascontiguousarray(np.stack([wv_all[v][b, qs] for v in range(4)]))
        m["idx"] = np.ascontiguousarray(idx_w)
        in_maps.append(m)
    return in_maps


def kernel(**inputs):
    key = NQ
    if key not in _prog_cache:
        _prog_cache[key] = build_program(NQ)
    nc = _prog_cache[key]
    in_maps = _host_prep(**inputs)
    res = run_bass_kernel_spmd(nc, in_maps, list(range(NCORES)))
    out = np.empty((B, Q, 3), np.float32)
    for core in range(NCORES):
        b, half = core // QSH, core % QSH
        out[b, half * NQ:(half + 1) * NQ, :] = res.results[core]["out_t"].T
    return out

